# revision 23
# baseline (speedup 1.0000x reference)
"""Type-2 NUFFT (image -> non-uniform k-space) on 8 Trainium2 NeuronCores.

kspace[b,m] = sum_{x,y} image[b,x,y] * exp(-i*(kx_m*(x-128) + ky_m*(y-128)))

Per core (M sharded 8 ways -> 2048 points):
  The image is folded even/odd along y (y'=y-128) into a concatenated rhs
  img_oe = [odd(129) | even(129)] so stage 1 is 2 fp32 matmul chains per
  (batch, m-tile):   S-chain: SxT.T @ img_oe -> [B_odd | B_even]
                     C-chain: CxT.T @ img_oe -> [A_odd | A_even]
  Stage 2 is one fused DVE multiply+row-reduce per output component using
  strided access patterns over the PSUM banks and a shared trig table
  W = [-Sy' | Cy | Sy']:
     Re[m]  = sum(B_odd*-Sy') + sum(A_even*Cy)
     -Im[m] = sum(B_even*Cy)  + sum(A_odd*Sy')

Trig tables on-chip: P = k*grid/(2pi); f = P - round(P) via the fp32
magic-constant trick; sin = Sin(2pi*f) on ScalarE (LUT valid on [-pi,pi]);
cos = 1 - 2*Sin(pi*f)^2.
"""

import sys

if '/opt/trn_rl_repo' not in sys.path:
    sys.path.insert(0, '/opt/trn_rl_repo')

import numpy as np

B, NX, NY, M, NCORES = 2, 256, 256, 16384, 8
ML = M // NCORES            # 2048 m-points per core
NT = ML // 128              # 16 m-tiles per core
TWO_PI = float(2.0 * np.pi)
PI = float(np.pi)
MAGIC = 12582912.0          # 1.5 * 2**23: (x + MAGIC) - MAGIC == round(x) fp32
NS = 129                    # one fold segment (incl pad/singles)
NSEG = 2 * NS               # 258: [odd | even] rhs width
NW = 3 * NS                 # 387: [-Sy' | Cy | Sy']

_CACHE = {}
USE_F32R = True


def _consts():
    xs = (np.arange(NX, dtype=np.float64) - NX // 2) / (2.0 * np.pi)
    xs_cols = xs.astype(np.float32).reshape(2, 128).T.copy()  # [128, 2]
    # y args (in "turns"): odd segment [1..127, -128, 0(pad)], even [0..128]
    ys_odd = np.concatenate([np.arange(1, 128), [-128.0], [0.0]])
    ys_even = np.arange(0, 129, dtype=np.float64)
    yargs = (np.concatenate([ys_odd, ys_even]) / (2.0 * np.pi)).astype(np.float32)
    ysb = np.broadcast_to(yargs.reshape(1, NSEG), (128, NSEG)).copy()
    ident16 = np.eye(16, dtype=np.float32)
    return xs_cols, ysb, ident16


def _build():
    import concourse.bacc as bacc
    import concourse.bass as bass
    import concourse.mybir as mybir
    from concourse.tile import TileContext

    A = mybir.AluOpType
    F = mybir.ActivationFunctionType
    f32 = mybir.dt.float32

    nc = bacc.Bacc("TRN2", target_bir_lowering=False, debug=False)

    image = nc.dram_tensor("image", [B, NX, NY], f32, kind="ExternalInput")
    traj = nc.dram_tensor("traj", [2, ML], f32, kind="ExternalInput")
    xs_cols = nc.dram_tensor("xs_cols", [128, 2], f32, kind="ExternalInput")
    ysb = nc.dram_tensor("ysb", [128, NSEG], f32, kind="ExternalInput")
    ident16 = nc.dram_tensor("ident16", [16, 16], f32, kind="ExternalInput")
    out = nc.dram_tensor("out", [128, 4 * NT], f32, kind="ExternalOutput")

    mmdt = mybir.dt.float32r if USE_F32R else f32

    def mmcast(ap):
        return ap

    def seg2(ap_tile, start, seg_stride):
        """[128, 2, NS] view: two NS-wide segments at start, start+seg_stride."""
        t_ = ap_tile.tensor
        row = ap_tile.ap[0][0]
        return bass.AP(t_, ap_tile.offset + start,
                       [[row, 128], [seg_stride, 2], [1, NS]])

    with TileContext(nc) as tc:
        with tc.tile_pool(name="const", bufs=1) as cpool, \
             tc.tile_pool(name="xtab", bufs=1) as xpool, \
             tc.tile_pool(name="xscratch", bufs=2) as xs_pool, \
             tc.tile_pool(name="ytab", bufs=10) as ypool, \
             tc.tile_pool(name="work", bufs=6) as wpool:

            # ---------------- constants / inputs ----------------
            xs_sb = cpool.tile([128, 2], f32)
            nc.sync.dma_start(xs_sb[:, :], xs_cols[:, :])
            kxb = cpool.tile([128, ML], f32)
            nc.sync.dma_start(kxb[:, :], traj[0:1, :].to_broadcast((128, ML)))
            ysb_sb = cpool.tile([128, NSEG], f32)
            nc.sync.dma_start(ysb_sb[:, :], ysb[:, :])
            id16 = cpool.tile([16, 16], f32)
            nc.sync.dma_start(id16[:, :], ident16[:, :])

            ky16 = cpool.tile([16, 128], f32)
            nc.sync.dma_start(
                ky16[:, :], traj[1:2, :].rearrange("o (t p) -> (o t) p", p=128))
            ky_col = cpool.tile([128, NT], f32)
            half_pi = cpool.tile([128, 1], f32)
            nc.vector.memset(half_pi[:, :], PI / 2.0)

            # image load + even/odd y-fold into concat rhs [odd | even]
            img_oe = {}
            for b in range(B):
                for k in range(2):
                    raw = wpool.tile([128, NY], f32, tag="imgraw")
                    nc.sync.dma_start(
                        raw[:, :], image[b, k * 128:(k + 1) * 128, :])
                    oe = cpool.tile([128, NSEG], mmdt, name=f"ioe_{b}_{k}")
                    # odd seg: cols 0:127 pairs, 127 = img[:,0], 128 = zero pad
                    nc.vector.tensor_sub(
                        oe[:, 0:127], raw[:, 129:256], raw[:, 127:0:-1])
                    nc.vector.tensor_copy(oe[:, 127:128], raw[:, 0:1])
                    nc.vector.tensor_scalar_mul(oe[:, 128:129], raw[:, 0:1], 0.0)
                    # even seg: col 129 = img[:,128], 130:257 pairs, 257 = img[:,0]
                    nc.vector.tensor_copy(oe[:, 129:130], raw[:, 128:129])
                    nc.vector.tensor_add(
                        oe[:, 130:257], raw[:, 129:256], raw[:, 127:0:-1])
                    nc.vector.tensor_copy(oe[:, 257:258], raw[:, 0:1])
                    img_oe[(b, k)] = oe

            # ---------------- x tables: CxT/SxT [x(2x128), m(2048)] --------
            cxT = [xpool.tile([128, ML], mmdt, name=f"cxT{h}") for h in range(2)]
            sxT = [xpool.tile([128, ML], mmdt, name=f"sxT{h}") for h in range(2)]
            with tc.tile_pool(name="psP", bufs=1, space="PSUM") as psP:
                ky_ps = psP.tile([128, 16], f32, tag="kyT")
                nc.tensor.transpose(ky_ps[:, :], ky16[:, :], id16[:, :])
                nc.scalar.copy(ky_col[:, :], ky_ps[:, :])
            for h in range(2):
                P = xs_pool.tile([128, ML], f32, tag="xP")
                nc.gpsimd.tensor_scalar(
                    P[:, :], kxb[:, :], scalar1=xs_sb[:, h:h + 1],
                    scalar2=None, op0=A.mult)
                rs = xs_pool.tile([128, ML], f32, tag="xrs")
                nc.gpsimd.tensor_scalar(
                    rs[:, :], P[:, :], scalar1=MAGIC, scalar2=MAGIC,
                    op0=A.add, op1=A.subtract)
                fs = xs_pool.tile([128, ML], f32, tag="xfs")
                nc.vector.scalar_tensor_tensor(
                    fs[:, :], P[:, :], 1.0, rs[:, :],
                    op0=A.mult, op1=A.subtract)
                nc.scalar.activation(
                    sxT[h][:, :], fs[:, :], F.Sin, scale=TWO_PI)
                fa = xs_pool.tile([128, ML], f32, tag="xfa")
                nc.scalar.activation(fa[:, :], fs[:, :], F.Abs)
                nc.scalar.activation(
                    cxT[h][:, :], fa[:, :], F.Sin, scale=-TWO_PI,
                    bias=half_pi[:, :])

            # ---------------- per m-tile main loop ----------------
            out_sb = cpool.tile([128, 4 * NT], f32)
            psAB_cm = tc.tile_pool(name="psAB", bufs=4, space="PSUM")
            psAB = psAB_cm.__enter__()
            for t in range(NT):
                # --- shared y table W = [-Sy'(129) | Cy(129) | Sy'(129)] ---
                u = ky_col[:, t:t + 1]
                p_y = ypool.tile([128, NSEG], f32, tag="py")
                nc.gpsimd.tensor_scalar(
                    p_y[:, :], ysb_sb[:, :], scalar1=u, scalar2=None, op0=A.mult)
                rs_y = ypool.tile([128, NSEG], f32, tag="yrs")
                nc.gpsimd.tensor_scalar(
                    rs_y[:, :], p_y[:, :], scalar1=MAGIC, scalar2=MAGIC,
                    op0=A.add, op1=A.subtract)
                fs_y = ypool.tile([128, NSEG], f32, tag="yfs")
                nc.vector.scalar_tensor_tensor(
                    fs_y[:, :], p_y[:, :], 1.0, rs_y[:, :],
                    op0=A.mult, op1=A.subtract)
                w = ypool.tile([128, NW], f32, tag="w")
                nc.scalar.activation(
                    w[:, 0:NS], fs_y[:, 0:NS], F.Sin, scale=-TWO_PI)
                nc.scalar.activation(
                    w[:, 2 * NS:NW], fs_y[:, 0:NS], F.Sin, scale=TWO_PI)
                fa_y = ypool.tile([128, NS], f32, tag="yfa")
                nc.scalar.activation(fa_y[:, :], fs_y[:, NS:NSEG], F.Abs)
                nc.scalar.activation(
                    w[:, NS:2 * NS], fa_y[:, :], F.Sin, scale=-TWO_PI,
                    bias=half_pi[:, :])

                for b in range(B):
                    # --- stage 1: bank0 = [B_odd|B_even], bank1 = [A_odd|A_even]
                    ab = psAB.tile([128, 1024], f32, tag="ab")
                    for k in range(2):
                        nc.tensor.matmul(
                            ab[:, 0:NSEG],
                            mmcast(sxT[k][:, t * 128:(t + 1) * 128]),
                            mmcast(img_oe[(b, k)][:, :]),
                            start=(k == 0), stop=(k == 1))
                    for k in range(2):
                        nc.tensor.matmul(
                            ab[:, 512:512 + NSEG],
                            mmcast(cxT[k][:, t * 128:(t + 1) * 128]),
                            mmcast(img_oe[(b, k)][:, :]),
                            start=(k == 0), stop=(k == 1))
                    # --- stage 2: fused multiply + row-reduce (strided APs) ---
                    # Re  = sum(B_odd * -Sy') + sum(A_even * Cy)
                    #       in0 segments at col 0 (B_odd), col 512+129 (A_even)
                    # -Im = sum(B_even * Cy) + sum(A_odd * Sy')
                    #       in0 segments at col 129 (B_even), col 512 (A_odd)
                    scr = wpool.tile([128, NSEG], f32, tag="scr")
                    scr2 = wpool.tile([128, NSEG], f32, tag="scr2")
                    col_re = (2 * b) * NT + t
                    col_im = (2 * b + 1) * NT + t
                    nc.vector.scalar_tensor_tensor(
                        seg2(scr, 0, NS), seg2(ab, 0, 641), 1.0,
                        seg2(w, 0, NS),
                        op0=A.mult, op1=A.mult,
                        accum_out=out_sb[:, col_re:col_re + 1])
                    nc.vector.scalar_tensor_tensor(
                        seg2(scr2, 0, NS), seg2(ab, NS, 383), 1.0,
                        seg2(w, NS, NS),
                        op0=A.mult, op1=A.mult,
                        accum_out=out_sb[:, col_im:col_im + 1])

            nc.sync.dma_start(out[:, :], out_sb[:, :])
            psAB_cm.__exit__(None, None, None)

    nc.compile()
    return nc


def kernel(image, trajectory):
    from concourse.bass_utils import run_bass_kernel_spmd

    if 'nc' not in _CACHE:
        _CACHE['nc'] = _build()
    nc = _CACHE['nc']

    image = np.ascontiguousarray(np.asarray(image, dtype=np.float32))
    trajectory = np.ascontiguousarray(np.asarray(trajectory, dtype=np.float32))
    xs_cols, ysb, ident16 = _consts()

    in_maps = []
    for c in range(NCORES):
        in_maps.append({
            "image": image,
            "traj": np.ascontiguousarray(trajectory[:, c * ML:(c + 1) * ML]),
            "xs_cols": xs_cols,
            "ysb": ysb,
            "ident16": ident16,
        })

    res = run_bass_kernel_spmd(nc, in_maps, core_ids=list(range(NCORES)))

    kspace = np.empty((B, M), dtype=np.complex64)
    for c in range(NCORES):
        o = res.results[c]["out"]          # [128, 4*NT]
        o = o.reshape(128, 2, 2, NT)       # [p, b, (re, -im), t]
        for b in range(B):
            re = o[:, b, 0, :].T.reshape(ML)   # m = t*128 + p
            im = -o[:, b, 1, :].T.reshape(ML)
            kspace[b, c * ML:(c + 1) * ML] = re + 1j * im
    return kspace


# revision 26
# speedup vs baseline: 5759.7714x; 5759.7714x over previous
"""Type-2 NUFFT (image -> non-uniform k-space) on 8 Trainium2 NeuronCores.

kspace[b,m] = sum_{x,y} image[b,x,y] * exp(-i*(kx_m*(x-128) + ky_m*(y-128)))

Per core (M sharded 8 ways -> 2048 points):
  The image is folded even/odd along y (y'=y-128) into a concatenated rhs
  img_oe = [odd(129) | even(129)] so stage 1 is 2 fp32 matmul chains per
  (batch, m-tile):   S-chain: SxT.T @ img_oe -> [B_odd | B_even]
                     C-chain: CxT.T @ img_oe -> [A_odd | A_even]
  Stage 2 is one fused DVE multiply+row-reduce per output component using
  strided access patterns over the PSUM banks and a shared trig table
  W = [-Sy' | Cy | Sy']:
     Re[m]  = sum(B_odd*-Sy') + sum(A_even*Cy)
     -Im[m] = sum(B_even*Cy)  + sum(A_odd*Sy')

Trig tables on-chip: P = k*grid/(2pi); f = P - round(P) via the fp32
magic-constant trick; sin = Sin(2pi*f) on ScalarE (LUT valid on [-pi,pi]);
cos(2pi*f) = Sin(-2pi*|f| + pi/2) (stays inside the LUT domain).
"""

import sys

if '/opt/trn_rl_repo' not in sys.path:
    sys.path.insert(0, '/opt/trn_rl_repo')

import numpy as np

B, NX, NY, M, NCORES = 2, 256, 256, 16384, 8
ML = M // NCORES            # 2048 m-points per core
NT = ML // 128              # 16 m-tiles per core
TWO_PI = float(2.0 * np.pi)
PI = float(np.pi)
MAGIC = 12582912.0          # 1.5 * 2**23: (x + MAGIC) - MAGIC == round(x) fp32
NS = 129                    # one fold segment (incl pad/singles)
NSEG = 2 * NS               # 258: [odd | even] rhs width
NW = 3 * NS                 # 387: [-Sy' | Cy | Sy']

_CACHE = {}
USE_F32R = True


def _consts():
    xs = (np.arange(NX, dtype=np.float64) - NX // 2) / (2.0 * np.pi)
    xs_cols = xs.astype(np.float32).reshape(2, 128).T.copy()  # [128, 2]
    # y args (in "turns"): odd segment [1..127, -128, 0(pad)], even [0..128]
    ys_odd = np.concatenate([np.arange(1, 128), [-128.0], [0.0]])
    ys_even = np.arange(0, 129, dtype=np.float64)
    yargs = (np.concatenate([ys_odd, ys_even]) / (2.0 * np.pi)).astype(np.float32)
    ysb = np.broadcast_to(yargs.reshape(1, NSEG), (128, NSEG)).copy()
    ident16 = np.eye(16, dtype=np.float32)
    return xs_cols, ysb, ident16


def _build():
    import concourse.bacc as bacc
    import concourse.bass as bass
    import concourse.mybir as mybir
    from concourse.tile import TileContext

    A = mybir.AluOpType
    F = mybir.ActivationFunctionType
    f32 = mybir.dt.float32

    nc = bacc.Bacc("TRN2", target_bir_lowering=False, debug=False)

    image = nc.dram_tensor("image", [B, NX, NY], f32, kind="ExternalInput")
    traj = nc.dram_tensor("traj", [2, ML], f32, kind="ExternalInput")
    xs_cols = nc.dram_tensor("xs_cols", [128, 2], f32, kind="ExternalInput")
    ysb = nc.dram_tensor("ysb", [128, NSEG], f32, kind="ExternalInput")
    ident16 = nc.dram_tensor("ident16", [16, 16], f32, kind="ExternalInput")
    out = nc.dram_tensor("out", [128, 4 * NT], f32, kind="ExternalOutput")

    mmdt = mybir.dt.float32r if USE_F32R else f32

    def mmcast(ap):
        return ap

    def seg2(ap_tile, start, seg_stride):
        """[128, 2, NS] view: two NS-wide segments at start, start+seg_stride."""
        t_ = ap_tile.tensor
        row = ap_tile.ap[0][0]
        return bass.AP(t_, ap_tile.offset + start,
                       [[row, 128], [seg_stride, 2], [1, NS]])

    with TileContext(nc) as tc:
        with tc.tile_pool(name="const", bufs=1) as cpool, \
             tc.tile_pool(name="xtab", bufs=1) as xpool, \
             tc.tile_pool(name="xscratch", bufs=2) as xs_pool, \
             tc.tile_pool(name="ytab", bufs=10) as ypool, \
             tc.tile_pool(name="work", bufs=6) as wpool:

            # ---------------- constants / inputs ----------------
            xs_sb = cpool.tile([128, 2], f32)
            nc.sync.dma_start(xs_sb[:, :], xs_cols[:, :])
            kxb = cpool.tile([128, ML], f32)
            nc.sync.dma_start(kxb[:, :], traj[0:1, :].to_broadcast((128, ML)))
            ysb_sb = cpool.tile([128, NSEG], f32)
            nc.sync.dma_start(ysb_sb[:, :], ysb[:, :])
            id16 = cpool.tile([16, 16], f32)
            nc.sync.dma_start(id16[:, :], ident16[:, :])

            ky16 = cpool.tile([16, 128], f32)
            nc.sync.dma_start(
                ky16[:, :], traj[1:2, :].rearrange("o (t p) -> (o t) p", p=128))
            ky_col = cpool.tile([128, NT], f32)
            half_pi = cpool.tile([128, 1], f32)
            nc.vector.memset(half_pi[:, :], PI / 2.0)

            # image load + even/odd y-fold into concat rhs [odd | even]
            img_oe = {}
            for b in range(B):
                for k in range(2):
                    raw = wpool.tile([128, NY], f32, tag="imgraw")
                    nc.sync.dma_start(
                        raw[:, :], image[b, k * 128:(k + 1) * 128, :])
                    oe = cpool.tile([128, NSEG], mmdt, name=f"ioe_{b}_{k}")
                    # odd seg: cols 0:127 pairs, 127 = img[:,0], 128 = zero pad
                    nc.vector.tensor_sub(
                        oe[:, 0:127], raw[:, 129:256], raw[:, 127:0:-1])
                    nc.scalar.copy(oe[:, 127:128], raw[:, 0:1])
                    nc.scalar.mul(oe[:, 128:129], raw[:, 0:1], 0.0)
                    # even seg: col 129 = img[:,128], 130:257 pairs, 257 = img[:,0]
                    nc.scalar.copy(oe[:, 129:130], raw[:, 128:129])
                    nc.vector.tensor_add(
                        oe[:, 130:257], raw[:, 129:256], raw[:, 127:0:-1])
                    nc.scalar.copy(oe[:, 257:258], raw[:, 0:1])
                    img_oe[(b, k)] = oe

            # ---------------- x tables: CxT/SxT [x(2x128), m(2048)] --------
            cxT = [xpool.tile([128, ML], mmdt, name=f"cxT{h}") for h in range(2)]
            sxT = [xpool.tile([128, ML], mmdt, name=f"sxT{h}") for h in range(2)]
            with tc.tile_pool(name="psP", bufs=1, space="PSUM") as psP:
                ky_ps = psP.tile([128, 16], f32, tag="kyT")
                nc.tensor.transpose(ky_ps[:, :], ky16[:, :], id16[:, :])
                nc.scalar.copy(ky_col[:, :], ky_ps[:, :])
            for h in range(2):
                P = xs_pool.tile([128, ML], f32, tag="xP")
                nc.gpsimd.tensor_scalar(
                    P[:, :], kxb[:, :], scalar1=xs_sb[:, h:h + 1],
                    scalar2=None, op0=A.mult)
                rs = xs_pool.tile([128, ML], f32, tag="xrs")
                nc.gpsimd.tensor_scalar(
                    rs[:, :], P[:, :], scalar1=MAGIC, scalar2=MAGIC,
                    op0=A.add, op1=A.subtract)
                fs = xs_pool.tile([128, ML], f32, tag="xfs")
                nc.vector.scalar_tensor_tensor(
                    fs[:, :], P[:, :], 1.0, rs[:, :],
                    op0=A.mult, op1=A.subtract)
                nc.scalar.activation(
                    sxT[h][:, :], fs[:, :], F.Sin, scale=TWO_PI)
                fa = xs_pool.tile([128, ML], f32, tag="xfa")
                nc.scalar.activation(fa[:, :], fs[:, :], F.Abs)
                nc.scalar.activation(
                    cxT[h][:, :], fa[:, :], F.Sin, scale=-TWO_PI,
                    bias=half_pi[:, :])

            # ---------------- per m-tile main loop ----------------
            out_sb = cpool.tile([128, 4 * NT], f32)
            psAB_cm = tc.tile_pool(name="psAB", bufs=4, space="PSUM")
            psAB = psAB_cm.__enter__()
            for t in range(NT):
                # --- shared y table W = [-Sy'(129) | Cy(129) | Sy'(129)] ---
                u = ky_col[:, t:t + 1]
                p_y = ypool.tile([128, NSEG], f32, tag="py")
                nc.gpsimd.tensor_scalar(
                    p_y[:, :], ysb_sb[:, :], scalar1=u, scalar2=None, op0=A.mult)
                rs_y = ypool.tile([128, NSEG], f32, tag="yrs")
                nc.gpsimd.tensor_scalar(
                    rs_y[:, :], p_y[:, :], scalar1=MAGIC, scalar2=MAGIC,
                    op0=A.add, op1=A.subtract)
                fs_y = ypool.tile([128, NSEG], f32, tag="yfs")
                nc.vector.scalar_tensor_tensor(
                    fs_y[:, :], p_y[:, :], 1.0, rs_y[:, :],
                    op0=A.mult, op1=A.subtract)
                w = ypool.tile([128, NW], f32, tag="w")
                nc.scalar.activation(
                    w[:, 0:NS], fs_y[:, 0:NS], F.Sin, scale=-TWO_PI)
                nc.scalar.activation(
                    w[:, 2 * NS:NW], fs_y[:, 0:NS], F.Sin, scale=TWO_PI)
                fa_y = ypool.tile([128, NS], f32, tag="yfa")
                nc.scalar.activation(fa_y[:, :], fs_y[:, NS:NSEG], F.Abs)
                nc.scalar.activation(
                    w[:, NS:2 * NS], fa_y[:, :], F.Sin, scale=-TWO_PI,
                    bias=half_pi[:, :])

                for b in range(B):
                    # --- stage 1: bank0 = [B_odd|B_even], bank1 = [A_odd|A_even]
                    ab = psAB.tile([128, 1024], f32, tag="ab")
                    for k in range(2):
                        nc.tensor.matmul(
                            ab[:, 0:NSEG],
                            mmcast(sxT[k][:, t * 128:(t + 1) * 128]),
                            mmcast(img_oe[(b, k)][:, :]),
                            start=(k == 0), stop=(k == 1))
                    for k in range(2):
                        nc.tensor.matmul(
                            ab[:, 512:512 + NSEG],
                            mmcast(cxT[k][:, t * 128:(t + 1) * 128]),
                            mmcast(img_oe[(b, k)][:, :]),
                            start=(k == 0), stop=(k == 1))
                    # --- stage 2: fused multiply + row-reduce (strided APs) ---
                    # Re  = sum(B_odd * -Sy') + sum(A_even * Cy)
                    #       in0 segments at col 0 (B_odd), col 512+129 (A_even)
                    # -Im = sum(B_even * Cy) + sum(A_odd * Sy')
                    #       in0 segments at col 129 (B_even), col 512 (A_odd)
                    scr = wpool.tile([128, NSEG], f32, tag="scr")
                    scr2 = wpool.tile([128, NSEG], f32, tag="scr2")
                    col_re = (2 * b) * NT + t
                    col_im = (2 * b + 1) * NT + t
                    nc.vector.scalar_tensor_tensor(
                        seg2(scr, 0, NS), seg2(ab, 0, 641), 1.0,
                        seg2(w, 0, NS),
                        op0=A.mult, op1=A.mult,
                        accum_out=out_sb[:, col_re:col_re + 1])
                    nc.vector.scalar_tensor_tensor(
                        seg2(scr2, 0, NS), seg2(ab, NS, 383), 1.0,
                        seg2(w, NS, NS),
                        op0=A.mult, op1=A.mult,
                        accum_out=out_sb[:, col_im:col_im + 1])

            nc.sync.dma_start(out[:, :], out_sb[:, :])
            psAB_cm.__exit__(None, None, None)

    nc.compile()
    return nc


def kernel(image, trajectory):
    from concourse.bass_utils import run_bass_kernel_spmd

    if 'nc' not in _CACHE:
        _CACHE['nc'] = _build()
    nc = _CACHE['nc']

    image = np.ascontiguousarray(np.asarray(image, dtype=np.float32))
    trajectory = np.ascontiguousarray(np.asarray(trajectory, dtype=np.float32))
    xs_cols, ysb, ident16 = _consts()

    in_maps = []
    for c in range(NCORES):
        in_maps.append({
            "image": image,
            "traj": np.ascontiguousarray(trajectory[:, c * ML:(c + 1) * ML]),
            "xs_cols": xs_cols,
            "ysb": ysb,
            "ident16": ident16,
        })

    res = run_bass_kernel_spmd(nc, in_maps, core_ids=list(range(NCORES)))

    kspace = np.empty((B, M), dtype=np.complex64)
    for c in range(NCORES):
        o = res.results[c]["out"]          # [128, 4*NT]
        o = o.reshape(128, 2, 2, NT)       # [p, b, (re, -im), t]
        for b in range(B):
            re = o[:, b, 0, :].T.reshape(ML)   # m = t*128 + p
            im = -o[:, b, 1, :].T.reshape(ML)
            kspace[b, c * ML:(c + 1) * ML] = re + 1j * im
    return kspace
